# revision 1
# baseline (speedup 1.0000x reference)
"""GQA attention (tanh-score + static bias, no softmax) on 8 trn2 cores.

Reference shapes: x [4,32,256,512], H=8 heads, G=2 kv groups, D=64, N=256.
Strategy: data-parallel over the 128 (b,t) pairs -> 16 per core, zero
collectives.  Per (b,t):
  q = x@Wq, k = x@Wk, v = x@Wv          (feature-major via host-transposed x)
  scores^T[m,n] = k_g @ q_h^T           (K=64 contraction, base-aligned)
  attn^T = tanh(scores^T * 0.125)       (ACT engine, scale fused)
  out_h^T = v_g^T @ attn_h^T + (sgr v_g)^T   (sgr@v once per group, add fused
                                              into the PSUM->SBUF evacuation)
  y = out @ Wo                           (Wo host-permuted to match pair order)

Host-side prep (outside the HW kernel): x transposed to feature-major and
pre-tiled, sgr transposed, Wk concatenated with its group-swapped copy (so
every head's score matmul finds its K block at the right partition offset),
Wo row-permuted.
"""

import os
import sys

import numpy as np

for _p in ("/opt/trn_rl_repo",):
    if _p not in sys.path and os.path.isdir(_p):
        sys.path.insert(0, _p)

import concourse.bass as bass
import concourse.tile as tile
from concourse import bacc, mybir
from concourse.bass_utils import run_bass_kernel_spmd

F32 = mybir.dt.float32
F32R = mybir.dt.float32r

B, T, N, C = 4, 32, 256, 512
H, G, D = 8, 2, 64
NCORES = 8
BT = B * T                      # 128
PER_CORE = BT // NCORES         # 16
NPAIR = PER_CORE // 2           # 8 iterations of 2 (b,t) each
SCALE = D ** -0.5               # 0.125

_cached = {}


def _build_nc():
    """Build + lower the single-core SPMD program."""
    nc = bacc.Bacc("TRN2", target_bir_lowering=False, debug=False,
                   num_devices=NCORES)

    # DRAM I/O (per-core shard, host-side pre-arranged)
    # xarr[i, p, c, 256*b + n] = x[bt=2i+b, tok=n, cin=128c+p]
    xT = nc.dram_tensor("xT", [NPAIR, 128, 4, 512], F32R, kind="ExternalInput").ap()
    sgrT = nc.dram_tensor("sgrT", [N, N], F32R, kind="ExternalInput").ap()
    Wq = nc.dram_tensor("Wq", [C, C], F32R, kind="ExternalInput").ap()
    Wkc = nc.dram_tensor("Wkc", [C, 2 * G * D], F32R, kind="ExternalInput").ap()
    Wv = nc.dram_tensor("Wv", [C, G * D], F32R, kind="ExternalInput").ap()
    Wop = nc.dram_tensor("Wop", [C, C], F32R, kind="ExternalInput").ap()
    y = nc.dram_tensor("y", [PER_CORE, N, C], F32, kind="ExternalOutput").ap()

    with tile.TileContext(nc) as tc:
        _body(tc, xT, sgrT, Wq, Wkc, Wv, Wop, y)

    nc.compile()
    return nc


def _body(tc, xT, sgrT, Wq, Wkc, Wv, Wop, y):
    nc = tc.nc

    def mm(out, lhsT, rhs, **kw):
        # float32r streams 1 row/cycle (vs 4 for fp32) at free dim >= 256.
        # All lhsT/rhs tiles are allocated as F32R so their producers round.
        nc.tensor.matmul(out, lhsT, rhs, **kw)
    import contextlib
    ctx = contextlib.ExitStack()
    with ctx:
        consts = ctx.enter_context(tc.tile_pool(name="consts", bufs=1))
        xpool = ctx.enter_context(tc.tile_pool(name="xt", bufs=2))
        qpool = ctx.enter_context(tc.tile_pool(name="qs", bufs=8))
        kpool = ctx.enter_context(tc.tile_pool(name="ks", bufs=4))
        vpool = ctx.enter_context(tc.tile_pool(name="vs", bufs=8))
        svpool = ctx.enter_context(tc.tile_pool(name="svs", bufs=4))
        apool = ctx.enter_context(tc.tile_pool(name="attn", bufs=34))
        ppool = ctx.enter_context(tc.tile_pool(name="pairs", bufs=10))
        ypool = ctx.enter_context(tc.tile_pool(name="ys", bufs=6))
        psA = ctx.enter_context(
            tc.tile_pool(name="psA", bufs=3, space=bass.MemorySpace.PSUM))
        psB = ctx.enter_context(
            tc.tile_pool(name="psB", bufs=5, space=bass.MemorySpace.PSUM))

        # ---- resident constants ----
        wq = []
        wkc = []
        wv = []
        wo = []
        for c in range(4):
            t = consts.tile([128, 512], F32R, tag=f"wq{c}")
            nc.sync.dma_start(t[:], Wq[128 * c:128 * (c + 1), :])
            wq.append(t)
            t = consts.tile([128, 256], F32R, tag=f"wkc{c}")
            nc.sync.dma_start(t[:], Wkc[128 * c:128 * (c + 1), :])
            wkc.append(t)
            t = consts.tile([128, 128], F32R, tag=f"wv{c}")
            nc.sync.dma_start(t[:], Wv[128 * c:128 * (c + 1), :])
            wv.append(t)
            t = consts.tile([128, 512], F32R, tag=f"wo{c}")
            nc.sync.dma_start(t[:], Wop[128 * c:128 * (c + 1), :])
            wo.append(t)
        sgt = []
        for mc in range(2):
            t = consts.tile([128, 256], F32R, tag=f"sgt{mc}")
            nc.sync.dma_start(t[:], sgrT[128 * mc:128 * (mc + 1), :])
            sgt.append(t)

        # per-iteration state handed from stage A to stage B
        state = [None] * NPAIR

        def stage_a(it):
            xt = xpool.tile([128, 4, 512], F32R, tag="xt")
            nc.sync.dma_start(xt[:], xT[it])

            # q projection: feature-major q^T, couts 128j..128j+127,
            # free = 512 (two bt's 256 tokens each)
            qs = []
            for j in range(4):
                ps = psA.tile([128, 512], F32, tag="psA")
                for c in range(4):
                    mm(ps[:], wq[c][:, 128 * j:128 * (j + 1)],
                                     xt[:, c, :], start=(c == 0), stop=(c == 3))
                s = qpool.tile([128, 512], F32R, tag="qs")
                nc.vector.tensor_copy(s[:], ps[:])
                qs.append(s)

            # k projections: k1 = [g0;g1] rows, k2 = [g1;g0] rows
            ks = []
            for jj in range(2):
                ps = psA.tile([128, 512], F32, tag="psA")
                for c in range(4):
                    mm(ps[:], wkc[c][:, 128 * jj:128 * (jj + 1)],
                                     xt[:, c, :], start=(c == 0), stop=(c == 3))
                s = kpool.tile([128, 512], F32R, tag="ks")
                nc.vector.tensor_copy(s[:], ps[:])
                ks.append(s)

            # v token-major: [tok 128, 128 (g0 d | g1 d)] per (b, tok-chunk)
            vs = [[None, None], [None, None]]
            for b in range(2):
                for mc in range(2):
                    ps = psB.tile([128, 128], F32, tag="psB")
                    off = 256 * b + 128 * mc
                    for c in range(4):
                        mm(ps[:], xt[:, c, off:off + 128],
                                         wv[c][:], start=(c == 0), stop=(c == 3))
                    s = vpool.tile([128, 128], F32R, tag="vs")
                    nc.vector.tensor_copy(s[:], ps[:])
                    vs[b][mc] = s

            # sgr @ v, both groups at once: lhsT = full v tile [m,128] so
            # out rows 0:64 = (sgr v_g0)^T, rows 64:128 = (sgr v_g1)^T and the
            # PSUM dst stays at partition base 0 (fp32r ISA constraint).
            svs = []
            for b in range(2):
                ps = psB.tile([128, 256], F32, tag="psB")
                for mc in range(2):
                    mm(ps[:], vs[b][mc][:], sgt[mc][:],
                       start=(mc == 0), stop=(mc == 1))
                s = svpool.tile([128, 256], F32, tag="svs")
                nc.vector.tensor_copy(s[:], ps[:])
                svs.append(s)

            state[it] = (qs, ks, vs, svs)

        def stage_b(it):
            qs, ks, vs, svs = state[it]
            # scores + tanh for both bt first (gives ACT a head start),
            # then attn@v + output projection per bt.
            attn = [[[None, None] for _ in range(H)] for _ in range(2)]
            for b in range(2):
                for h in range(H):
                    half = h % 2            # row half of q tile / PE array
                    grp = h // 4
                    # pick the k layout whose needed group sits in `half`
                    ksrc = ks[0] if (grp == half == 0 or grp == half == 1) \
                        else ks[1]
                    # rows of both operands at base partition 64*half
                    r0, r1 = 64 * half, 64 * (half + 1)
                    rhs = qs[h // 2][r0:r1, 256 * b:256 * (b + 1)]
                    for mc in range(2):
                        off = 256 * b + 128 * mc
                        ps = psB.tile([128, 256], F32, tag="psB")
                        mm(ps[:], ksrc[r0:r1, off:off + 128],
                                         rhs, start=True, stop=True)
                        a = apool.tile([128, 256], F32R, tag="attn")
                        nc.scalar.activation(
                            a[:], ps[:], mybir.ActivationFunctionType.Tanh,
                            scale=SCALE)
                        attn[b][h][mc] = a

            for b in range(2):
                pairs = []
                for p in range(4):
                    # full-v lhsT: head h's matmul yields (attn_h @ v_g0)^T in
                    # rows 0:64 and (attn_h @ v_g1)^T in rows 64:128; keep the
                    # half belonging to h's group.  PSUM dst base stays 0.
                    psl = psB.tile([128, 256], F32, tag="psB")
                    psh = psB.tile([128, 256], F32, tag="psB")
                    for mc in range(2):
                        mm(psl[:], vs[b][mc][:], attn[b][p][mc][:],
                           start=(mc == 0), stop=(mc == 1))
                    for mc in range(2):
                        mm(psh[:], vs[b][mc][:], attn[b][p + 4][mc][:],
                           start=(mc == 0), stop=(mc == 1))
                    s = ppool.tile([128, 256], F32R, tag="pairs")
                    nc.vector.tensor_add(s[0:64, :], psl[0:64, :],
                                         svs[b][0:64, :])
                    nc.vector.tensor_add(s[64:128, :], psh[64:128, :],
                                         svs[b][64:128, :])
                    pairs.append(s)

                for tc_ in range(2):
                    ps = psA.tile([128, 512], F32, tag="psA")
                    for p in range(4):
                        mm(ps[:],
                                         pairs[p][:, 128 * tc_:128 * (tc_ + 1)],
                                         wo[p][:], start=(p == 0), stop=(p == 3))
                    s = ypool.tile([128, 512], F32, tag="ys")
                    nc.vector.tensor_copy(s[:], ps[:])
                    nc.sync.dma_start(
                        y[2 * it + b, 128 * tc_:128 * (tc_ + 1), :], s[:])
            state[it] = None

        # 1-deep software pipeline: stage A of iter i+1 is emitted (and thus
        # sits in the PE queue) before stage B of iter i, so projections of
        # the next pair overlap the tanh/attention tail of the current one.
        stage_a(0)
        for it in range(NPAIR):
            if it + 1 < NPAIR:
                stage_a(it + 1)
            stage_b(it)


def _get_runner():
    if "nc" not in _cached:
        _cached["nc"] = _build_nc()
    return _cached["nc"]


def _prep_inputs(x, sgr, Wq, Wk, Wv, Wo):
    x = np.ascontiguousarray(x, dtype=np.float32)
    xb = x.reshape(BT, N, C)
    # Wk with groups swapped, concatenated
    Wk = np.asarray(Wk, dtype=np.float32)
    Wkc = np.concatenate([Wk, np.concatenate([Wk[:, D:], Wk[:, :D]], axis=1)],
                         axis=1)
    # Wo rows permuted to pair order [h0,h4 | h1,h5 | h2,h6 | h3,h7]
    perm = np.concatenate(
        [np.r_[64 * p:64 * (p + 1), 64 * (p + 4):64 * (p + 5)]
         for p in range(4)])
    Wop = np.ascontiguousarray(np.asarray(Wo, dtype=np.float32)[perm, :])
    sgrT = np.ascontiguousarray(np.asarray(sgr, dtype=np.float32).T)
    Wq = np.ascontiguousarray(np.asarray(Wq, dtype=np.float32))
    Wv = np.ascontiguousarray(np.asarray(Wv, dtype=np.float32))

    in_maps = []
    for core in range(NCORES):
        xc = xb[PER_CORE * core: PER_CORE * (core + 1)]        # [16, 256, 512]
        xtc = xc.transpose(0, 2, 1)                            # [16, 512, 256]
        xarr = np.ascontiguousarray(
            xtc.reshape(NPAIR, 2, 4, 128, N)
               .transpose(0, 3, 2, 1, 4)
               .reshape(NPAIR, 128, 4, 512))
        in_maps.append({
            "xT": xarr, "sgrT": sgrT, "Wq": Wq, "Wkc": Wkc,
            "Wv": Wv, "Wop": Wop,
        })
    return in_maps


def _run(x, sgr, Wq, Wk, Wv, Wo, trace=False, tmpdir=None):
    nc = _get_runner()
    in_maps = _prep_inputs(x, sgr, Wq, Wk, Wv, Wo)
    res = run_bass_kernel_spmd(nc, in_maps, list(range(NCORES)), trace=trace,
                               tmpdir=tmpdir)
    outs = [res.results[i]["y"] for i in range(NCORES)]
    full = np.concatenate(outs, axis=0).reshape(B, T, N, C)
    return full, res


def kernel(x, sgr, Wq, Wk, Wv, Wo):
    out, _ = _run(x, sgr, Wq, Wk, Wv, Wo, trace=False)
    return out



# revision 3
# speedup vs baseline: 1.4213x; 1.4213x over previous
"""GQA attention (tanh-score + static bias, no softmax) on 8 trn2 cores.

Reference shapes: x [4,32,256,512], H=8 heads, G=2 kv groups, D=64, N=256.
Strategy: data-parallel over the 128 (b,t) pairs -> 16 per core, zero
collectives.  All matmul operands are bf16 (PSUM accumulation stays fp32):
bf16 streams 1 row/cycle at any free size (fp32r needs free>=256), enables
the fast weight-load path, and halves SBUF/HBM traffic.  Per (b,t):
  q = x@Wq, k = x@Wk, v = x@Wv          (feature-major via host-transposed x)
  scores^T[m,n] = k_g @ q_h^T           (K=64 contraction, base-aligned;
                                         head pair (h, h+4) shares one PSUM
                                         bank -> one 512-wide tanh ACTIVATE)
  attn^T = tanh(scores^T * 0.125)       (ACT engine, scale fused, bf16 out)
  out_h^T = v_g^T @ attn_h^T + (sgr v_g)^T   (one 512-wide matmul per head
                                              pair; sgr@v once per group,
                                              added during PSUM evacuation)
  y = out @ Wo                           (Wo host-permuted to match pair order)

Emission interleaves next-iteration projections between score/attention units
so the PE never idles waiting on the ACT (tanh) pipeline.

Host-side prep (outside the HW kernel): x transposed to feature-major,
pre-tiled and cast to bf16, sgr transposed, Wk concatenated with its
group-swapped copy, Wo row-permuted; all weights cast to bf16.
"""

import os
import sys

import numpy as np

for _p in ("/opt/trn_rl_repo",):
    if _p not in sys.path and os.path.isdir(_p):
        sys.path.insert(0, _p)

import ml_dtypes

import concourse.bass as bass
import concourse.tile as tile
from concourse import bacc, mybir
from concourse.bass_utils import run_bass_kernel_spmd

F32 = mybir.dt.float32
BF16 = mybir.dt.bfloat16

B, T, N, C = 4, 32, 256, 512
H, G, D = 8, 2, 64
NCORES = 8
BT = B * T                      # 128
PER_CORE = BT // NCORES         # 16
NPAIR = PER_CORE // 2           # 8 iterations of 2 (b,t) each
SCALE = D ** -0.5               # 0.125

_cached = {}


def _build_nc():
    """Build + lower the single-core SPMD program."""
    nc = bacc.Bacc("TRN2", target_bir_lowering=False, debug=False,
                   num_devices=NCORES)

    # DRAM I/O (per-core shard, host-side pre-arranged)
    # xarr[i, p, c, 256*b + n] = x[bt=2i+b, tok=n, cin=128c+p]
    xT = nc.dram_tensor("xT", [NPAIR, 128, 4, 512], BF16, kind="ExternalInput").ap()
    sgrT = nc.dram_tensor("sgrT", [N, N], BF16, kind="ExternalInput").ap()
    Wq = nc.dram_tensor("Wq", [C, C], BF16, kind="ExternalInput").ap()
    Wkc = nc.dram_tensor("Wkc", [C, 2 * G * D], BF16, kind="ExternalInput").ap()
    Wv = nc.dram_tensor("Wv", [C, G * D], BF16, kind="ExternalInput").ap()
    Wop = nc.dram_tensor("Wop", [C, C], BF16, kind="ExternalInput").ap()
    y = nc.dram_tensor("y", [PER_CORE, N, C], F32, kind="ExternalOutput").ap()

    with tile.TileContext(nc) as tc:
        _body(tc, xT, sgrT, Wq, Wkc, Wv, Wop, y)

    nc.compile()
    return nc


def _body(tc, xT, sgrT, Wq, Wkc, Wv, Wop, y):
    nc = tc.nc
    mm = nc.tensor.matmul
    import contextlib
    ctx = contextlib.ExitStack()
    with ctx:
        consts = ctx.enter_context(tc.tile_pool(name="consts", bufs=1))
        xpool = ctx.enter_context(tc.tile_pool(name="xt", bufs=3))
        qpool = ctx.enter_context(tc.tile_pool(name="qs", bufs=8))
        kpool = ctx.enter_context(tc.tile_pool(name="ks", bufs=4))
        vpool = ctx.enter_context(tc.tile_pool(name="vs", bufs=2))
        svpool = ctx.enter_context(tc.tile_pool(name="svs", bufs=2))
        apool = ctx.enter_context(tc.tile_pool(name="attn", bufs=18))
        ppool = ctx.enter_context(tc.tile_pool(name="pairs", bufs=10))
        ypool = ctx.enter_context(tc.tile_pool(name="ys", bufs=6))
        psA = ctx.enter_context(
            tc.tile_pool(name="psA", bufs=2, space=bass.MemorySpace.PSUM))
        psS = ctx.enter_context(
            tc.tile_pool(name="psS", bufs=3, space=bass.MemorySpace.PSUM))
        psV = ctx.enter_context(
            tc.tile_pool(name="psV", bufs=3, space=bass.MemorySpace.PSUM))

        # ---- resident constants ----
        wq = []
        wkc = []
        wv = []
        wo = []
        for c in range(4):
            t = consts.tile([128, 512], BF16, tag=f"wq{c}")
            nc.sync.dma_start(t[:], Wq[128 * c:128 * (c + 1), :])
            wq.append(t)
            t = consts.tile([128, 256], BF16, tag=f"wkc{c}")
            nc.sync.dma_start(t[:], Wkc[128 * c:128 * (c + 1), :])
            wkc.append(t)
            t = consts.tile([128, 128], BF16, tag=f"wv{c}")
            nc.sync.dma_start(t[:], Wv[128 * c:128 * (c + 1), :])
            wv.append(t)
            t = consts.tile([128, 512], BF16, tag=f"wo{c}")
            nc.sync.dma_start(t[:], Wop[128 * c:128 * (c + 1), :])
            wo.append(t)
        sgt = []
        for mc in range(2):
            t = consts.tile([128, 256], BF16, tag=f"sgt{mc}")
            nc.sync.dma_start(t[:], sgrT[128 * mc:128 * (mc + 1), :])
            sgt.append(t)

        # pipeline state: projections of the iteration currently in stage B
        # (cur) and the one being produced by interleaved stage A (new)
        xts = [None] * NPAIR

        def dma_x(it):
            if it < NPAIR:
                t = xpool.tile([128, 4, 512], BF16, tag="xt")
                nc.sync.dma_start(t[:], xT[it])
                xts[it] = t

        # ---- stage A units (projections for iteration `it`) ----
        def make_a_units(it):
            xt = xts[it]
            qs_new = [None] * 4
            ks_new = [None] * 2
            vsv = {}

            def qu(j):
                ps = psA.tile([128, 512], F32, tag="psA")
                for c in range(4):
                    mm(ps[:], wq[c][:, 128 * j:128 * (j + 1)],
                       xt[:, c, :], start=(c == 0), stop=(c == 3))
                s = qpool.tile([128, 512], BF16, tag="qs")
                nc.vector.tensor_copy(s[:], ps[:])
                qs_new[j] = s

            def ku(jj):
                ps = psA.tile([128, 512], F32, tag="psA")
                for c in range(4):
                    mm(ps[:], wkc[c][:, 128 * jj:128 * (jj + 1)],
                       xt[:, c, :], start=(c == 0), stop=(c == 3))
                s = kpool.tile([128, 512], BF16, tag="ks")
                nc.vector.tensor_copy(s[:], ps[:])
                ks_new[jj] = s

            def vu(k):
                # v token-major quarters: k = 2*b + mc -> [128 tok, 128 dd]
                if k == 0:
                    vsv["ps"] = psV.tile([128, 512], F32, tag="psV", name="vps")
                ps = vsv["ps"]
                b, mc = k // 2, k % 2
                off = 256 * b + 128 * mc
                for c in range(4):
                    mm(ps[:, 128 * k:128 * (k + 1)], xt[:, c, off:off + 128],
                       wv[c][:], start=(c == 0), stop=(c == 3))
                if k == 3:
                    s = vpool.tile([128, 512], BF16, tag="vs")
                    nc.vector.tensor_copy(s[:], ps[:])
                    vsv["vs"] = s

            def sgru():
                # (sgr @ v)^T for both groups, per b: cols 256*b..256*b+255
                vs_new = vsv["vs"]
                ps = psV.tile([128, 512], F32, tag="psV")
                for b in range(2):
                    for mc in range(2):
                        mm(ps[:, 256 * b:256 * (b + 1)],
                           vs_new[:, 128 * (2 * b + mc):128 * (2 * b + mc + 1)],
                           sgt[mc][:], start=(mc == 0), stop=(mc == 1))
                s = svpool.tile([128, 512], F32, tag="svs")
                nc.vector.tensor_copy(s[:], ps[:])
                vsv["svs"] = s

            return qu, ku, vu, sgru, qs_new, ks_new, vsv

        state = [None] * (NPAIR + 1)

        def stage_a_plain(it):
            qu, ku, vu, sgru, qs_new, ks_new, vsv = make_a_units(it)
            for j in range(4):
                qu(j)
            for jj in range(2):
                ku(jj)
            for k in range(4):
                vu(k)
            sgru()
            state[it] = (qs_new, ks_new, vsv)

        def emit_iter(it):
            """Stage B of iteration `it`, interleaved with stage A of it+1."""
            qs, ks, vsv_cur = state[it]
            vs_cur = vsv_cur["vs"]
            svs_cur = vsv_cur["svs"]
            has_next = it + 1 < NPAIR
            dma_x(it + 2)
            if has_next:
                qu, ku, vu, sgru, qs_new, ks_new, vsv_new = make_a_units(it + 1)
                big = [lambda j=j: qu(j) for j in range(4)] + \
                      [lambda jj=jj: ku(jj) for jj in range(2)]
                small = [lambda k=k: vu(k) for k in range(4)] + [sgru]
            else:
                big = [lambda: None] * 6
                small = [lambda: None] * 5

            attn = [[[None, None] for _ in range(4)] for _ in range(2)]
            pairs = [[None] * 4 for _ in range(2)]

            def su(i):
                # scores^T + tanh for head pair (p, p+4): one PSUM bank,
                # one 512-wide ACTIVATE
                b, p, mc = i // 8, (i % 8) // 2, i % 2
                half = p % 2
                r0, r1 = 64 * half, 64 * (half + 1)
                off = 256 * b + 128 * mc
                ps = psS.tile([128, 512], F32, tag="psS")
                mm(ps[:, 0:256], ks[half][r0:r1, off:off + 128],
                   qs[p // 2][r0:r1, 256 * b:256 * (b + 1)],
                   start=True, stop=True)
                mm(ps[:, 256:512], ks[1 - half][r0:r1, off:off + 128],
                   qs[p // 2 + 2][r0:r1, 256 * b:256 * (b + 1)],
                   start=True, stop=True)
                a = apool.tile([128, 512], BF16, tag="attn")
                nc.scalar.activation(
                    a[:], ps[:], mybir.ActivationFunctionType.Tanh,
                    scale=SCALE)
                attn[b][p][mc] = a

            def av(j):
                # (attn @ v)^T for the head pair, both m-chunks accumulated;
                # rows 0:64 x cols 0:256 belong to head p (group 0), rows
                # 64:128 x cols 256:512 to head p+4 (group 1).  sgr@v is
                # added during the PSUM->SBUF evacuation.
                b, p = j // 4, j % 4
                ps = psV.tile([128, 512], F32, tag="psV")
                for mc in range(2):
                    mm(ps[:], vs_cur[:, 128 * (2 * b + mc):128 * (2 * b + mc + 1)],
                       attn[b][p][mc][:], start=(mc == 0), stop=(mc == 1))
                s = ppool.tile([128, 256], BF16, tag="pairs")
                nc.vector.tensor_add(s[0:64, :], ps[0:64, 0:256],
                                     svs_cur[0:64, 256 * b:256 * (b + 1)])
                nc.vector.tensor_add(s[64:128, :], ps[64:128, 256:512],
                                     svs_cur[64:128, 256 * b:256 * (b + 1)])
                pairs[b][p] = s

            def out(b, tc_):
                ps = psA.tile([128, 512], F32, tag="psA")
                for p in range(4):
                    mm(ps[:], pairs[b][p][:, 128 * tc_:128 * (tc_ + 1)],
                       wo[p][:], start=(p == 0), stop=(p == 3))
                s = ypool.tile([128, 512], F32, tag="ys")
                nc.vector.tensor_copy(s[:], ps[:])
                nc.sync.dma_start(
                    y[2 * it + b, 128 * tc_:128 * (tc_ + 1), :], s[:])

            # interleaved emission: scores feed ACT early; projection matmuls
            # of it+1 fill the PE while ACT drains; attn@v follows tanh.
            for i in range(6):
                su(i)
                big[i]()
            su(6); av(0)
            su(7); small[0]()
            su(8); av(1)
            su(9); small[1]()
            su(10); av(2)
            su(11); small[2]()
            su(12); av(3)
            su(13); small[3]()
            su(14); small[4]()
            su(15)
            av(4); av(5); av(6); av(7)
            out(0, 0); out(0, 1); out(1, 0); out(1, 1)

            state[it] = None
            if has_next:
                state[it + 1] = (qs_new, ks_new, vsv_new)

        dma_x(0)
        dma_x(1)
        stage_a_plain(0)
        for it in range(NPAIR):
            emit_iter(it)


def _get_runner():
    if "nc" not in _cached:
        _cached["nc"] = _build_nc()
    return _cached["nc"]


def _prep_inputs(x, sgr, Wq, Wk, Wv, Wo):
    bf16 = ml_dtypes.bfloat16
    x = np.ascontiguousarray(x, dtype=np.float32)
    xb = x.reshape(BT, N, C)
    # Wk with groups swapped, concatenated
    Wk = np.asarray(Wk, dtype=np.float32)
    Wkc = np.concatenate([Wk, np.concatenate([Wk[:, D:], Wk[:, :D]], axis=1)],
                         axis=1).astype(bf16)
    # Wo rows permuted to pair order [h0,h4 | h1,h5 | h2,h6 | h3,h7]
    perm = np.concatenate(
        [np.r_[64 * p:64 * (p + 1), 64 * (p + 4):64 * (p + 5)]
         for p in range(4)])
    Wop = np.ascontiguousarray(
        np.asarray(Wo, dtype=np.float32)[perm, :]).astype(bf16)
    sgrT = np.ascontiguousarray(
        np.asarray(sgr, dtype=np.float32).T).astype(bf16)
    Wq = np.ascontiguousarray(np.asarray(Wq, dtype=np.float32)).astype(bf16)
    Wv = np.ascontiguousarray(np.asarray(Wv, dtype=np.float32)).astype(bf16)

    in_maps = []
    for core in range(NCORES):
        xc = xb[PER_CORE * core: PER_CORE * (core + 1)]        # [16, 256, 512]
        xtc = xc.transpose(0, 2, 1)                            # [16, 512, 256]
        xarr = np.ascontiguousarray(
            xtc.reshape(NPAIR, 2, 4, 128, N)
               .transpose(0, 3, 2, 1, 4)
               .reshape(NPAIR, 128, 4, 512)).astype(bf16)
        in_maps.append({
            "xT": xarr, "sgrT": sgrT, "Wq": Wq, "Wkc": Wkc,
            "Wv": Wv, "Wop": Wop,
        })
    return in_maps


def _run(x, sgr, Wq, Wk, Wv, Wo, trace=False, tmpdir=None):
    nc = _get_runner()
    in_maps = _prep_inputs(x, sgr, Wq, Wk, Wv, Wo)
    res = run_bass_kernel_spmd(nc, in_maps, list(range(NCORES)), trace=trace,
                               tmpdir=tmpdir)
    outs = [res.results[i]["y"] for i in range(NCORES)]
    full = np.concatenate(outs, axis=0).reshape(B, T, N, C)
    return full, res


def kernel(x, sgr, Wq, Wk, Wv, Wo):
    out, _ = _run(x, sgr, Wq, Wk, Wv, Wo, trace=False)
    return out


# revision 11
# speedup vs baseline: 1.4458x; 1.0173x over previous
"""GQA attention (tanh-score + static bias, no softmax) on 8 trn2 cores.

Reference shapes: x [4,32,256,512], H=8 heads, G=2 kv groups, D=64, N=256.
Strategy: data-parallel over the 128 (b,t) pairs -> 16 per core, zero
collectives.  All matmul operands are bf16 (PSUM accumulation stays fp32):
bf16 streams 1 row/cycle at any free size (fp32r needs free>=256), enables
the fast weight-load path, and halves SBUF/HBM traffic.  Per (b,t):
  q = x@Wq, k = x@Wk, v = x@Wv          (feature-major via host-transposed x)
  scores^T[m,n] = k_g @ q_h^T           (K=64 contraction, base-aligned;
                                         head pair (h, h+4) shares one PSUM
                                         bank -> one 512-wide tanh ACTIVATE)
  attn^T = tanh(scores^T * 0.125)       (ACT engine, scale fused, bf16 out)
  out_h^T = v_g^T @ attn_h^T + (sgr v_g)^T   (one 512-wide matmul per head
                                              pair; sgr@v once per group,
                                              added during PSUM evacuation)
  y = out @ Wo                           (Wo host-permuted to match pair order)

Emission interleaves next-iteration projections between score/attention units
so the PE never idles waiting on the ACT (tanh) pipeline.

Host-side prep (outside the HW kernel): x transposed to feature-major,
pre-tiled and cast to bf16, sgr transposed, Wk concatenated with its
group-swapped copy, Wo row-permuted; all weights cast to bf16.
"""

import os
import sys

import numpy as np

for _p in ("/opt/trn_rl_repo",):
    if _p not in sys.path and os.path.isdir(_p):
        sys.path.insert(0, _p)

import ml_dtypes

import concourse.bass as bass
import concourse.tile as tile
from concourse import bacc, mybir
from concourse.bass_utils import run_bass_kernel_spmd

F32 = mybir.dt.float32
BF16 = mybir.dt.bfloat16

B, T, N, C = 4, 32, 256, 512
H, G, D = 8, 2, 64
NCORES = 8
BT = B * T                      # 128
PER_CORE = BT // NCORES         # 16
NPAIR = PER_CORE // 2           # 8 iterations of 2 (b,t) each
SCALE = D ** -0.5               # 0.125

_cached = {}


def _build_nc():
    """Build + lower the single-core SPMD program."""
    nc = bacc.Bacc("TRN2", target_bir_lowering=False, debug=False,
                   num_devices=NCORES)

    # DRAM I/O (per-core shard, host-side pre-arranged)
    # xarr[i, p, c, 256*b + n] = x[bt=2i+b, tok=n, cin=128c+p]
    xT = nc.dram_tensor("xT", [NPAIR, 128, 4, 512], BF16, kind="ExternalInput").ap()
    sgrT = nc.dram_tensor("sgrT", [N, N], BF16, kind="ExternalInput").ap()
    Wq = nc.dram_tensor("Wq", [C, C], BF16, kind="ExternalInput").ap()
    Wkc = nc.dram_tensor("Wkc", [C, G * D], BF16, kind="ExternalInput").ap()
    Wv = nc.dram_tensor("Wv", [C, G * D], BF16, kind="ExternalInput").ap()
    Wop = nc.dram_tensor("Wop", [C, C], BF16, kind="ExternalInput").ap()
    y = nc.dram_tensor("y", [PER_CORE, N, C], F32, kind="ExternalOutput").ap()

    with tile.TileContext(nc) as tc:
        _body(tc, xT, sgrT, Wq, Wkc, Wv, Wop, y)

    nc.compile()
    return nc


def _body(tc, xT, sgrT, Wq, Wkc, Wv, Wop, y):
    nc = tc.nc
    mm = nc.tensor.matmul
    import contextlib
    ctx = contextlib.ExitStack()
    with ctx:
        consts = ctx.enter_context(tc.tile_pool(name="consts", bufs=1))
        xpool = ctx.enter_context(tc.tile_pool(name="xt", bufs=3))
        qpool = ctx.enter_context(tc.tile_pool(name="qs", bufs=8))
        kpool = ctx.enter_context(tc.tile_pool(name="ks", bufs=4))
        vpool = ctx.enter_context(tc.tile_pool(name="vs", bufs=2))
        svpool = ctx.enter_context(tc.tile_pool(name="svs", bufs=2))
        apool = ctx.enter_context(tc.tile_pool(name="attn", bufs=18))
        ppool = ctx.enter_context(tc.tile_pool(name="pairs", bufs=10))
        ypool = ctx.enter_context(tc.tile_pool(name="ys", bufs=6))
        psA = ctx.enter_context(
            tc.tile_pool(name="psA", bufs=2, space=bass.MemorySpace.PSUM))
        psS = ctx.enter_context(
            tc.tile_pool(name="psS", bufs=3, space=bass.MemorySpace.PSUM))
        psV = ctx.enter_context(
            tc.tile_pool(name="psV", bufs=3, space=bass.MemorySpace.PSUM))

        # pipeline state: projections of the iteration currently in stage B
        # (cur) and the one being produced by interleaved stage A (new)
        xts = [None] * NPAIR

        def dma_x(it):
            if it < NPAIR:
                t = xpool.tile([128, 4, 512], BF16, tag="xt")
                nc.sync.dma_start(t[:], xT[it])
                xts[it] = t

        # ---- resident constants (x0 first so stage A can start ASAP; Wo
        # and sgr are not needed until well into the first iteration) ----
        dma_x(0)
        wq = []
        wkc = []
        wv = []
        wo = []
        for c in range(4):
            t = consts.tile([128, 512], BF16, tag=f"wq{c}")
            nc.sync.dma_start(t[:], Wq[128 * c:128 * (c + 1), :])
            wq.append(t)
            t = consts.tile([128, 128], BF16, tag=f"wkc{c}")
            nc.sync.dma_start(t[:], Wkc[128 * c:128 * (c + 1), :])
            wkc.append(t)
            t = consts.tile([128, 128], BF16, tag=f"wv{c}")
            nc.sync.dma_start(t[:], Wv[128 * c:128 * (c + 1), :])
            wv.append(t)
        dma_x(1)
        sgt = []
        for mc in range(2):
            t = consts.tile([128, 256], BF16, tag=f"sgt{mc}")
            nc.sync.dma_start(t[:], sgrT[128 * mc:128 * (mc + 1), :])
            sgt.append(t)
        for c in range(4):
            t = consts.tile([128, 512], BF16, tag=f"wo{c}")
            nc.sync.dma_start(t[:], Wop[128 * c:128 * (c + 1), :])
            wo.append(t)

        # ---- stage A units (projections for iteration `it`) ----
        def make_a_units(it):
            xt = xts[it]
            qs_new = [None] * 4
            ks_new = [None] * 2
            vsv = {}

            def qu(j):
                ps = psA.tile([128, 512], F32, tag="psA")
                for c in range(4):
                    mm(ps[:], wq[c][:, 128 * j:128 * (j + 1)],
                       xt[:, c, :], start=(c == 0), stop=(c == 3))
                s = qpool.tile([128, 512], BF16, tag="qs")
                nc.vector.tensor_copy(s[:], ps[:])
                qs_new[j] = s

            def ku():
                # k feature-major [g0 d; g1 d] x 512 tokens; the group-swapped
                # layout ks2 = [g1; g0] is a partition-swap SBUF->SBUF DMA
                # copy instead of a second projection (saves 4 matmuls).
                ps = psA.tile([128, 512], F32, tag="psA")
                for c in range(4):
                    mm(ps[:], wkc[c][:],
                       xt[:, c, :], start=(c == 0), stop=(c == 3))
                s = kpool.tile([128, 512], BF16, tag="ks")
                nc.vector.tensor_copy(s[:], ps[:])
                s2 = kpool.tile([128, 512], BF16, tag="ks")
                nc.sync.dma_start(s2[0:64, :], s[64:128, :])
                nc.sync.dma_start(s2[64:128, :], s[0:64, :])
                ks_new[0] = s
                ks_new[1] = s2

            def vu(k):
                # v token-major quarters: k = 2*b + mc -> [128 tok, 128 dd]
                if k == 0:
                    vsv["ps"] = psV.tile([128, 512], F32, tag="psV", name="vps")
                ps = vsv["ps"]
                b, mc = k // 2, k % 2
                off = 256 * b + 128 * mc
                for c in range(4):
                    mm(ps[:, 128 * k:128 * (k + 1)], xt[:, c, off:off + 128],
                       wv[c][:], start=(c == 0), stop=(c == 3))
                if k == 3:
                    s = vpool.tile([128, 512], BF16, tag="vs")
                    nc.vector.tensor_copy(s[:], ps[:])
                    vsv["vs"] = s

            def sgru():
                # (sgr @ v)^T for both groups, per b: cols 256*b..256*b+255
                vs_new = vsv["vs"]
                ps = psV.tile([128, 512], F32, tag="psV")
                for b in range(2):
                    for mc in range(2):
                        mm(ps[:, 256 * b:256 * (b + 1)],
                           vs_new[:, 128 * (2 * b + mc):128 * (2 * b + mc + 1)],
                           sgt[mc][:], start=(mc == 0), stop=(mc == 1))
                s = svpool.tile([128, 512], F32, tag="svs")
                nc.vector.tensor_copy(s[:], ps[:])
                vsv["svs"] = s

            return qu, ku, vu, sgru, qs_new, ks_new, vsv

        state = [None] * (NPAIR + 1)

        def stage_a_plain(it):
            qu, ku, vu, sgru, qs_new, ks_new, vsv = make_a_units(it)
            for j in range(4):
                qu(j)
            ku()
            for k in range(4):
                vu(k)
            sgru()
            state[it] = (qs_new, ks_new, vsv)

        def emit_iter(it):
            """Stage B of iteration `it`, interleaved with stage A of it+1."""
            qs, ks, vsv_cur = state[it]
            vs_cur = vsv_cur["vs"]
            svs_cur = vsv_cur["svs"]
            has_next = it + 1 < NPAIR
            dma_x(it + 2)
            if has_next:
                qu, ku, vu, sgru, qs_new, ks_new, vsv_new = make_a_units(it + 1)
                big = [lambda j=j: qu(j) for j in range(4)] + [ku, lambda: None]
                small = [lambda k=k: vu(k) for k in range(4)] + [sgru]

            attn = [[[None, None] for _ in range(4)] for _ in range(2)]
            pairs = [[None] * 4 for _ in range(2)]

            def su(i):
                # scores^T + tanh for head pair (p, p+4): one PSUM bank,
                # one 512-wide ACTIVATE
                b, p, mc = i // 8, (i % 8) // 2, i % 2
                half = p % 2
                r0, r1 = 64 * half, 64 * (half + 1)
                off = 256 * b + 128 * mc
                ps = psS.tile([128, 512], F32, tag="psS")
                mm(ps[:, 0:256], ks[half][r0:r1, off:off + 128],
                   qs[p // 2][r0:r1, 256 * b:256 * (b + 1)],
                   start=True, stop=True)
                mm(ps[:, 256:512], ks[1 - half][r0:r1, off:off + 128],
                   qs[p // 2 + 2][r0:r1, 256 * b:256 * (b + 1)],
                   start=True, stop=True)
                a = apool.tile([128, 512], BF16, tag="attn")
                nc.scalar.activation(
                    a[:], ps[:], mybir.ActivationFunctionType.Tanh,
                    scale=SCALE)
                attn[b][p][mc] = a

            def av(j):
                # (attn @ v)^T for the head pair, both m-chunks accumulated;
                # rows 0:64 x cols 0:256 belong to head p (group 0), rows
                # 64:128 x cols 256:512 to head p+4 (group 1).  sgr@v is
                # added during the PSUM->SBUF evacuation.
                b, p = j // 4, j % 4
                ps = psV.tile([128, 512], F32, tag="psV")
                for mc in range(2):
                    mm(ps[:], vs_cur[:, 128 * (2 * b + mc):128 * (2 * b + mc + 1)],
                       attn[b][p][mc][:], start=(mc == 0), stop=(mc == 1))
                s = ppool.tile([128, 256], BF16, tag="pairs")
                nc.vector.tensor_add(s[0:64, :], ps[0:64, 0:256],
                                     svs_cur[0:64, 256 * b:256 * (b + 1)])
                nc.vector.tensor_add(s[64:128, :], ps[64:128, 256:512],
                                     svs_cur[64:128, 256 * b:256 * (b + 1)])
                pairs[b][p] = s

            def out(b, tc_):
                ps = psA.tile([128, 512], F32, tag="psA")
                for p in range(4):
                    mm(ps[:], pairs[b][p][:, 128 * tc_:128 * (tc_ + 1)],
                       wo[p][:], start=(p == 0), stop=(p == 3))
                s = ypool.tile([128, 512], F32, tag="ys")
                nc.scalar.copy(s[:], ps[:])
                nc.sync.dma_start(
                    y[2 * it + b, 128 * tc_:128 * (tc_ + 1), :], s[:])

            # interleaved emission: scores feed ACT early; projection matmuls
            # of it+1 fill the PE while ACT drains; attn@v follows tanh.
            if has_next:
                for i in range(6):
                    su(i)
                    big[i]()
                su(6); av(0)
                su(7); small[0]()
                su(8); av(1)
                su(9); small[1]()
                su(10); av(2)
                su(11); small[2]()
                su(12); av(3)
                su(13); small[3]()
                su(14); small[4]()
                su(15)
                av(4); av(5); av(6); av(7)
                out(0, 0); out(0, 1); out(1, 0); out(1, 1)
            else:
                # tail: no stage-A filler; interleave attn@v and out-proj
                # between score units so the PE tracks the ACT pipeline.
                su(0); su(1); su(2); su(3); su(4); su(5)
                av(0); su(6)
                av(1); su(7)
                av(2); su(8)
                av(3); su(9)
                out(0, 0); su(10)
                out(0, 1); su(11)
                av(4); su(12)
                av(5); su(13)
                av(6); su(14)
                su(15); av(7)
                out(1, 0); out(1, 1)

            state[it] = None
            if has_next:
                state[it + 1] = (qs_new, ks_new, vsv_new)

        dma_x(0)
        dma_x(1)
        stage_a_plain(0)
        for it in range(NPAIR):
            emit_iter(it)


def _get_runner():
    if "nc" not in _cached:
        _cached["nc"] = _build_nc()
    return _cached["nc"]


def _prep_inputs(x, sgr, Wq, Wk, Wv, Wo):
    bf16 = ml_dtypes.bfloat16
    x = np.ascontiguousarray(x, dtype=np.float32)
    xb = x.reshape(BT, N, C)
    Wkc = np.ascontiguousarray(np.asarray(Wk, dtype=np.float32)).astype(bf16)
    # Wo rows permuted to pair order [h0,h4 | h1,h5 | h2,h6 | h3,h7]
    perm = np.concatenate(
        [np.r_[64 * p:64 * (p + 1), 64 * (p + 4):64 * (p + 5)]
         for p in range(4)])
    Wop = np.ascontiguousarray(
        np.asarray(Wo, dtype=np.float32)[perm, :]).astype(bf16)
    sgrT = np.ascontiguousarray(
        np.asarray(sgr, dtype=np.float32).T).astype(bf16)
    Wq = np.ascontiguousarray(np.asarray(Wq, dtype=np.float32)).astype(bf16)
    Wv = np.ascontiguousarray(np.asarray(Wv, dtype=np.float32)).astype(bf16)

    in_maps = []
    for core in range(NCORES):
        xc = xb[PER_CORE * core: PER_CORE * (core + 1)]        # [16, 256, 512]
        xtc = xc.transpose(0, 2, 1)                            # [16, 512, 256]
        xarr = np.ascontiguousarray(
            xtc.reshape(NPAIR, 2, 4, 128, N)
               .transpose(0, 3, 2, 1, 4)
               .reshape(NPAIR, 128, 4, 512)).astype(bf16)
        in_maps.append({
            "xT": xarr, "sgrT": sgrT, "Wq": Wq, "Wkc": Wkc,
            "Wv": Wv, "Wop": Wop,
        })
    return in_maps


def _run(x, sgr, Wq, Wk, Wv, Wo, trace=False, tmpdir=None):
    nc = _get_runner()
    in_maps = _prep_inputs(x, sgr, Wq, Wk, Wv, Wo)
    res = run_bass_kernel_spmd(nc, in_maps, list(range(NCORES)), trace=trace,
                               tmpdir=tmpdir)
    outs = [res.results[i]["y"] for i in range(NCORES)]
    full = np.concatenate(outs, axis=0).reshape(B, T, N, C)
    return full, res


def kernel(x, sgr, Wq, Wk, Wv, Wo):
    out, _ = _run(x, sgr, Wq, Wk, Wv, Wo, trace=False)
    return out
